# revision 1
# baseline (speedup 1.0000x reference)
"""ConcatCritic all-pairs MLP kernel for 8 trn2 NeuronCores.

final[p, q] = MLP(concat(x[p], y[q])) for B=1024 pairs each way;
MLP layers 128->128->128->64->64->64->1, relu on hidden layers.

Sharding: core d owns y rows [d*128, (d+1)*128) and all of x, producing a
[128, 1024] block S_d[qi, p] = g(x[p], y[d*128+qi]); the host concatenates
to S [1024, 1024] and returns S.T.

Per-core dataflow (feature-major: features on partitions, pair-rows stream
as the matmul moving dimension; all matmuls fp16 inputs / fp32 accumulate):
  - Layer 0 factorizes: concat(x,y) @ W0 = x @ W0[:64] + y @ W0[64:].
    U = (x @ W0x + b0)^T [128, 1024] and V = (y_d @ W0y)^T [128, 128] are
    computed once; per qi, h0 = relu(U + V[:, qi]) is one DVE tensor_scalar.
  - L1 (128->128): 2 matmuls N=512 -> psum [128, 1024]; ACT relu -> h1 fp16.
  - L2 (128->64): two col-tiled matmuls pack rows pairwise into one psum
    bank: [0:64, c] = row of each even 256-block, [64:128, c] = +256 row ->
    packed [128, 512]; DVE relu -> h2.
  - L3, L4 (64->64): block-diagonal [128, 128] weights keep the packed
    layout at full array width; DVE relu / ACT relu -> h3, h4.
  - L5 (64->1): [128, 32] zero-padded weights at col positions 0/32/64/96
    for 4 consecutive qi -> psum [128, 512]; one ACT copy (+b5) per 4 qi;
    one descrambling DMA per qi to S[qi, :].
"""
import os
import sys

sys.path.insert(0, "/opt/trn_rl_repo")

import numpy as np
from contextlib import ExitStack

import concourse.bass as bass
import concourse.mybir as mybir
import concourse.tile as tile
from concourse import bacc
from concourse.bass_utils import run_bass_kernel_spmd

F32 = mybir.dt.float32
FP16 = mybir.dt.float16
F32R = mybir.dt.float32r
AF = mybir.ActivationFunctionType
ALU = mybir.AluOpType

B = 1024
DX = 64
DY = 64
NCORES = 8
QPC = B // NCORES  # 128 y-rows per core
REPEAT = int(os.environ.get("KERNEL_REPEAT", "1"))
QPC_EFF = int(os.environ.get("KERNEL_QPC", str(QPC)))

_cache = {}


def round_f32r(a):
    """Round fp32 ndarray to the f32r grid (11 explicit mantissa bits, RNE)."""
    u = np.ascontiguousarray(a, dtype=np.float32).view(np.uint32)
    low = u & np.uint32(0x00000FFF)
    base = u & np.uint32(0xFFFFF000)
    lsb = (u >> np.uint32(12)) & np.uint32(1)
    round_up = (low > 0x800) | ((low == 0x800) & (lsb == 1))
    return (base + (round_up.astype(np.uint32) << np.uint32(12))).view(np.float32)


def build_nc():
    nc = bacc.Bacc("TRN2", target_bir_lowering=False, debug=False)

    d_xT = nc.dram_tensor("xT", [DX, B], F32R, kind="ExternalInput")
    d_yT = nc.dram_tensor("yT", [DY, QPC], F32R, kind="ExternalInput")
    d_w0x = nc.dram_tensor("w0x", [DX, 128], F32R, kind="ExternalInput")
    d_w0y = nc.dram_tensor("w0y", [DY, 128], F32R, kind="ExternalInput")
    d_w1 = nc.dram_tensor("w1", [128, 128], F32R, kind="ExternalInput")
    d_w2 = nc.dram_tensor("w2", [128, 64], FP16, kind="ExternalInput")
    d_w3p = nc.dram_tensor("w3p", [128, 128], F32R, kind="ExternalInput")
    d_w4p = nc.dram_tensor("w4p", [128, 128], F32R, kind="ExternalInput")
    d_w5p = nc.dram_tensor("w5p", [128, 2], F32R, kind="ExternalInput")
    d_b0 = nc.dram_tensor("b0", [128], F32, kind="ExternalInput")
    d_b1 = nc.dram_tensor("b1", [128], F32, kind="ExternalInput")
    d_b2p = nc.dram_tensor("b2p", [128], F32, kind="ExternalInput")
    d_b3p = nc.dram_tensor("b3p", [128], F32, kind="ExternalInput")
    d_b4p = nc.dram_tensor("b4p", [128], F32, kind="ExternalInput")
    d_b5 = nc.dram_tensor("b5", [128], F32, kind="ExternalInput")
    d_out = nc.dram_tensor("out", [QPC, B], F32, kind="ExternalOutput")

    with tile.TileContext(nc) as tc, ExitStack() as ctx:
        const = ctx.enter_context(tc.tile_pool(name="const", bufs=1))
        sb = ctx.enter_context(tc.tile_pool(name="sb", bufs=2))
        ps1 = ctx.enter_context(tc.tile_pool(name="ps1", bufs=2, space="PSUM"))
        psm = ctx.enter_context(tc.tile_pool(name="psm", bufs=2, space="PSUM"))

        # ---- load constants -------------------------------------------------
        xT = const.tile([DX, B], F32R)
        yT = const.tile([DY, QPC], F32R)
        w0x = const.tile([DX, 128], F32R)
        w0y = const.tile([DY, 128], F32R)
        w1 = const.tile([128, 128], F32R)
        w2 = const.tile([128, 64], FP16)
        w3p = const.tile([128, 128], F32R)
        w4p = const.tile([128, 128], F32R)
        w5p = const.tile([128, 2], F32R)
        b0 = const.tile([128, 1], F32)
        b1 = const.tile([128, 1], F32)
        b2p = const.tile([128, 1], F32)
        b3p = const.tile([128, 1], F32)
        b4p = const.tile([128, 1], F32)
        b5 = const.tile([128, 1], F32)
        for t, d in [(xT, d_xT), (yT, d_yT), (w0x, d_w0x), (w0y, d_w0y),
                     (w1, d_w1), (w2, d_w2), (w3p, d_w3p), (w4p, d_w4p),
                     (w5p, d_w5p)]:
            nc.sync.dma_start(t[:], d.ap())
        for t, d in [(b0, d_b0), (b1, d_b1), (b2p, d_b2p), (b3p, d_b3p),
                     (b4p, d_b4p), (b5, d_b5)]:
            nc.sync.dma_start(t[:], d.ap()[:, None])

        # ---- preamble: U = (x @ W0x + b0)^T fp16, V = (y @ W0y)^T f32 ------
        U = const.tile([128, B], F32R)
        V = const.tile([128, QPC], F32)
        pU = ps1.tile([128, B], F32, tag="p1")
        nc.tensor.matmul(pU[:, 0:512], w0x[:], xT[:, 0:512])
        nc.tensor.matmul(pU[:, 512:1024], w0x[:], xT[:, 512:1024])
        nc.scalar.activation(U[:], pU[:], AF.Identity, bias=b0[:], scale=1.0)
        pV = psm.tile([128, QPC], F32, tag="pm")
        nc.tensor.matmul(pV[:], w0y[:], yT[:])
        nc.scalar.copy(V[:], pV[:])

        # ---- main loop: batches of 2 qi ------------------------------------
        for _rep in range(REPEAT):
            for t in range(QPC_EFF // 2):
                qa = 2 * t
                # h0 = relu(U + V[:, qi]) per qi, then L1 + relu1 per qi
                h1s = []
                for qk in range(2):
                    qi = qa + qk
                    h0 = sb.tile([128, B], F32R, tag="h0")
                    nc.vector.tensor_scalar(
                        h0[:], U[:], V[:, qi:qi + 1], 0.0, ALU.add, ALU.max)
                    p1 = ps1.tile([128, B], F32, tag="p1")
                    nc.tensor.matmul(p1[:, 0:512], w1[:], h0[:, 0:512])
                    nc.tensor.matmul(p1[:, 512:1024], w1[:], h0[:, 512:1024])
                    h1 = sb.tile([128, B], FP16, tag="h1")
                    nc.scalar.activation(h1[:], p1[:], AF.Relu, bias=b1[:],
                                         scale=1.0)
                    h1s.append(h1)
                # L2 fp16 col-tiled pack, both qi into one [128, 1024] psum:
                # cols [qk*512 + j*256 + c] <- h1 row j*512 + (half? 256:0) + c
                p2 = psm.tile([128, B], F32, tag="pm")
                for half in range(2):
                    for qk in range(2):
                        h1v = h1s[qk][:].rearrange("p (j h c) -> p j h c",
                                                   j=2, h=2)
                        nc.tensor.matmul(
                            p2[64 * half:64 * half + 64,
                               512 * qk:512 * qk + 512]
                            .rearrange("p (j c) -> p j c", j=2),
                            w2[:], h1v[:, :, half:half + 1, :])
                h2 = sb.tile([128, B], F32R, tag="h2")
                nc.vector.tensor_scalar(
                    h2[:], p2[:], b2p[:], 0.0, ALU.add, ALU.max)
                # L3 block-diag f32r over both qi
                p3 = psm.tile([128, B], F32, tag="pm")
                nc.tensor.matmul(p3[:, 0:512], w3p[:], h2[:, 0:512])
                nc.tensor.matmul(p3[:, 512:1024], w3p[:], h2[:, 512:1024])
                h3 = sb.tile([128, B], F32R, tag="h3")
                nc.vector.tensor_scalar(
                    h3[:], p3[:], b3p[:], 0.0, ALU.add, ALU.max)
                # L4 block-diag f32r
                p4 = psm.tile([128, B], F32, tag="pm")
                nc.tensor.matmul(p4[:, 0:512], w4p[:], h3[:, 0:512])
                nc.tensor.matmul(p4[:, 512:1024], w4p[:], h3[:, 512:1024])
                h4 = sb.tile([128, B], F32R, tag="h4")
                nc.scalar.activation(h4[:], p4[:], AF.Relu, bias=b4p[:],
                                     scale=1.0)
                # L5 f32r [128, 2] -> [2, 1024]: partition 0 = top rows,
                # partition 1 = +256 rows; cols split by qi
                p5 = psm.tile([2, B], F32, tag="pm")
                nc.tensor.matmul(p5[:, 0:512], w5p[:], h4[:, 0:512])
                nc.tensor.matmul(p5[:, 512:1024], w5p[:], h4[:, 512:1024])
                stage = sb.tile([2, B], F32, tag="stage")
                nc.scalar.activation(stage[:], p5[:], AF.Identity,
                                     bias=b5[0:2, :], scale=1.0)
                # one DMA, scrambled layout: out_raw[qi, p*512 + j*256 + c]
                src = stage[0:2, :].rearrange("p (k c) -> p k c", k=2)
                dst = d_out.ap()[qa:qa + 2, :] \
                    .rearrange("k (p c) -> p k c", p=2)
                nc.sync.dma_start(dst, src)

    nc.compile()
    return nc


def make_in_maps(**inputs):
    x = np.asarray(inputs["x"], dtype=np.float32)
    y = np.asarray(inputs["y"], dtype=np.float32)
    Ws = [np.asarray(inputs[f"W{i}"], dtype=np.float32) for i in range(6)]
    bs = [np.asarray(inputs[f"b{i}"], dtype=np.float32) for i in range(6)]

    w3p = np.zeros((128, 128), np.float32)
    w3p[0:64, 0:64] = Ws[3]
    w3p[64:128, 64:128] = Ws[3]
    w4p = np.zeros((128, 128), np.float32)
    w4p[0:64, 0:64] = Ws[4]
    w4p[64:128, 64:128] = Ws[4]
    w5p = np.zeros((128, 2), np.float32)
    w5p[0:64, 0] = Ws[5][:, 0]
    w5p[64:128, 1] = Ws[5][:, 0]

    base = {
        "xT": round_f32r(x.T),
        "w0x": round_f32r(Ws[0][0:DX]),
        "w0y": round_f32r(Ws[0][DX:]),
        "w1": round_f32r(Ws[1]),
        "w2": Ws[2].astype(np.float16),
        "w3p": round_f32r(w3p),
        "w4p": round_f32r(w4p),
        "w5p": round_f32r(w5p),
        "b0": bs[0],
        "b1": bs[1],
        "b2p": np.concatenate([bs[2], bs[2]]),
        "b3p": np.concatenate([bs[3], bs[3]]),
        "b4p": np.concatenate([bs[4], bs[4]]),
        "b5": np.full(128, bs[5][0], np.float32),
    }
    in_maps = []
    for c in range(NCORES):
        m = dict(base)
        m["yT"] = round_f32r(y[c * QPC:(c + 1) * QPC].T)
        in_maps.append(m)
    return in_maps


def kernel(**inputs):
    in_maps = make_in_maps(**inputs)
    if "nc" not in _cache:
        _cache["nc"] = build_nc()
    res = None
    for attempt in range(3):
        try:
            res = run_bass_kernel_spmd(_cache["nc"], in_maps,
                                       core_ids=list(range(NCORES)))
            break
        except Exception:
            # transient NRT_EXEC_UNIT_UNRECOVERABLE wedges recover on retry
            if attempt == 2:
                raise
            import time
            time.sleep(5)
    raw = np.concatenate([res.results[c]["out"] for c in range(NCORES)], axis=0)
    S = raw.reshape(B, 2, 2, 256).transpose(0, 2, 1, 3).reshape(B, B)
    return np.ascontiguousarray(S.T)


if __name__ == "__main__":
    rng = np.random.default_rng(0)
    inputs = {"x": rng.standard_normal((B, DX), dtype=np.float32),
              "y": rng.standard_normal((B, DY), dtype=np.float32)}
    dims = [DX + DY, 128, 128, 64, 64, 64, 1]
    for i in range(6):
        s = np.sqrt(2.0 / (dims[i] + dims[i + 1])).astype(np.float32)
        inputs[f"W{i}"] = rng.standard_normal((dims[i], dims[i + 1]),
                                              dtype=np.float32) * s
        inputs[f"b{i}"] = rng.standard_normal(dims[i + 1]).astype(np.float32) * 0.1
    out = kernel(**inputs)
    h = np.concatenate([np.broadcast_to(inputs["x"][None], (B, B, DX)),
                        np.broadcast_to(inputs["y"][:, None], (B, B, DY))],
                       axis=2).reshape(B * B, DX + DY)
    for i in range(6):
        h = h @ inputs[f"W{i}"] + inputs[f"b{i}"]
        if i < 5:
            h = np.maximum(h, 0)
    ref = h.reshape(B, B).T
    err = np.abs(out - ref).max() / np.abs(ref).max()
    print(f"self-check relerr: {err:.3e}")



# revision 4
# speedup vs baseline: 22.5324x; 22.5324x over previous
"""ConcatCritic all-pairs MLP kernel for 8 trn2 NeuronCores.

final[p, q] = MLP(concat(x[p], y[q])) for B=1024 pairs each way;
MLP layers 128->128->128->64->64->64->1, relu on hidden layers.

Sharding: core d owns y rows [d*128, (d+1)*128) and all of x, producing a
[128, 1024] block S_d[qi, p] = g(x[p], y[d*128+qi]); the host concatenates
to S [1024, 1024] and returns S.T.

Per-core dataflow (feature-major: features on partitions, pair-rows stream
as the matmul moving dimension; all matmuls fp16 inputs / fp32 accumulate):
  - Layer 0 factorizes: concat(x,y) @ W0 = x @ W0[:64] + y @ W0[64:].
    U = (x @ W0x + b0)^T [128, 1024] and V = (y_d @ W0y)^T [128, 128] are
    computed once; per qi, h0 = relu(U + V[:, qi]) is one DVE tensor_scalar.
  - L1 (128->128): 2 matmuls N=512 -> psum [128, 1024]; ACT relu -> h1 fp16.
  - L2 (128->64): two col-tiled matmuls pack rows pairwise into one psum
    bank: [0:64, c] = row of each even 256-block, [64:128, c] = +256 row ->
    packed [128, 512]; DVE relu -> h2.
  - L3, L4 (64->64): block-diagonal [128, 128] weights keep the packed
    layout at full array width; DVE relu / ACT relu -> h3, h4.
  - L5 (64->1): [128, 32] zero-padded weights at col positions 0/32/64/96
    for 4 consecutive qi -> psum [128, 512]; one ACT copy (+b5) per 4 qi;
    one descrambling DMA per qi to S[qi, :].
"""
import os
import sys

sys.path.insert(0, "/opt/trn_rl_repo")

import numpy as np
from contextlib import ExitStack

import concourse.bass as bass
import concourse.mybir as mybir
import concourse.tile as tile
from concourse import bacc
from concourse.bass import ds
from concourse.bass_utils import run_bass_kernel_spmd

F32 = mybir.dt.float32
FP16 = mybir.dt.float16
F32R = mybir.dt.float32r
AF = mybir.ActivationFunctionType
ALU = mybir.AluOpType

B = 1024
DX = 64
DY = 64
NCORES = 8
QPC = B // NCORES  # 128 y-rows per core
REPEAT = int(os.environ.get("KERNEL_REPEAT", "1"))
QPC_EFF = int(os.environ.get("KERNEL_QPC", str(QPC)))
USE_LOOP = int(os.environ.get("KERNEL_LOOP", "1"))

_cache = {}


def round_f32r(a):
    """Round fp32 ndarray to the f32r grid (11 explicit mantissa bits, RNE)."""
    u = np.ascontiguousarray(a, dtype=np.float32).view(np.uint32)
    low = u & np.uint32(0x00000FFF)
    base = u & np.uint32(0xFFFFF000)
    lsb = (u >> np.uint32(12)) & np.uint32(1)
    round_up = (low > 0x800) | ((low == 0x800) & (lsb == 1))
    return (base + (round_up.astype(np.uint32) << np.uint32(12))).view(np.float32)


def build_nc():
    nc = bacc.Bacc("TRN2", target_bir_lowering=False, debug=False)

    d_xT = nc.dram_tensor("xT", [DX, B], F32R, kind="ExternalInput")
    d_yT = nc.dram_tensor("yT", [DY, QPC], F32R, kind="ExternalInput")
    d_w0x = nc.dram_tensor("w0x", [DX, 128], F32R, kind="ExternalInput")
    d_w0y = nc.dram_tensor("w0y", [DY, 128], F32R, kind="ExternalInput")
    d_w1 = nc.dram_tensor("w1", [128, 128], F32R, kind="ExternalInput")
    d_w2 = nc.dram_tensor("w2", [128, 64], FP16, kind="ExternalInput")
    d_w3p = nc.dram_tensor("w3p", [128, 128], F32R, kind="ExternalInput")
    d_w4p = nc.dram_tensor("w4p", [128, 128], F32R, kind="ExternalInput")
    d_w5p = nc.dram_tensor("w5p", [128, 2], F32R, kind="ExternalInput")
    d_b0 = nc.dram_tensor("b0", [128], F32, kind="ExternalInput")
    d_b1 = nc.dram_tensor("b1", [128], F32, kind="ExternalInput")
    d_b2p = nc.dram_tensor("b2p", [128], F32, kind="ExternalInput")
    d_b3p = nc.dram_tensor("b3p", [128], F32, kind="ExternalInput")
    d_b4p = nc.dram_tensor("b4p", [128], F32, kind="ExternalInput")
    d_b5 = nc.dram_tensor("b5", [128], F32, kind="ExternalInput")
    d_out = nc.dram_tensor("out", [QPC, B], F32, kind="ExternalOutput")

    with tile.TileContext(nc) as tc, ExitStack() as ctx:
        const = ctx.enter_context(tc.tile_pool(name="const", bufs=1))
        sb = ctx.enter_context(tc.tile_pool(name="sb", bufs=2))
        ps1 = ctx.enter_context(tc.tile_pool(name="ps1", bufs=2, space="PSUM"))
        psm = ctx.enter_context(tc.tile_pool(name="psm", bufs=2, space="PSUM"))

        # ---- load constants -------------------------------------------------
        xT = const.tile([DX, B], F32R)
        yT = const.tile([DY, QPC], F32R)
        w0x = const.tile([DX, 128], F32R)
        w0y = const.tile([DY, 128], F32R)
        w1 = const.tile([128, 128], F32R)
        w2 = const.tile([128, 64], FP16)
        w3p = const.tile([128, 128], F32R)
        w4p = const.tile([128, 128], F32R)
        w5p = const.tile([128, 2], F32R)
        b0 = const.tile([128, 1], F32)
        b1 = const.tile([128, 1], F32)
        b2p = const.tile([128, 1], F32)
        b3p = const.tile([128, 1], F32)
        b4p = const.tile([128, 1], F32)
        b5 = const.tile([128, 1], F32)
        for t, d in [(xT, d_xT), (yT, d_yT), (w0x, d_w0x), (w0y, d_w0y),
                     (w1, d_w1), (w2, d_w2), (w3p, d_w3p), (w4p, d_w4p),
                     (w5p, d_w5p)]:
            nc.sync.dma_start(t[:], d.ap())
        for t, d in [(b0, d_b0), (b1, d_b1), (b2p, d_b2p), (b3p, d_b3p),
                     (b4p, d_b4p), (b5, d_b5)]:
            nc.sync.dma_start(t[:], d.ap()[:, None])

        # ---- preamble: U = (x @ W0x + b0)^T fp16, V = (y @ W0y)^T f32 ------
        U = const.tile([128, B], F32R)
        V = const.tile([128, QPC], F32)
        pU = ps1.tile([128, B], F32, tag="p1")
        nc.tensor.matmul(pU[:, 0:512], w0x[:], xT[:, 0:512])
        nc.tensor.matmul(pU[:, 512:1024], w0x[:], xT[:, 512:1024])
        nc.scalar.activation(U[:], pU[:], AF.Identity, bias=b0[:], scale=1.0)
        pV = psm.tile([128, QPC], F32, tag="pm")
        nc.tensor.matmul(pV[:], w0y[:], yT[:])
        nc.scalar.copy(V[:], pV[:])

        # ---- main loop: batches of 2 qi ------------------------------------
        def body(qa):
            """qa: first qi of the batch; int (unrolled) or ScalarValue."""
            # h0 = relu(U + V[:, qi]) per qi, then L1 + relu1 per qi
            h1s = []
            for qk in range(2):
                h0 = sb.tile([128, B], F32R, tag="h0")
                nc.vector.tensor_scalar(
                    h0[:], U[:], V[:, ds(qa + qk, 1)], 0.0, ALU.add, ALU.max)
                p1 = ps1.tile([128, B], F32, tag="p1")
                nc.tensor.matmul(p1[:, 0:512], w1[:], h0[:, 0:512])
                nc.tensor.matmul(p1[:, 512:1024], w1[:], h0[:, 512:1024])
                h1 = sb.tile([128, B], FP16, tag="h1")
                nc.scalar.activation(h1[:], p1[:], AF.Relu, bias=b1[:],
                                     scale=1.0)
                h1s.append(h1)
            # L2 fp16 col-tiled pack, both qi into one [128, 1024] psum:
            # cols [qk*512 + j*256 + c] <- h1 row j*512 + (half? 256:0) + c
            p2 = psm.tile([128, B], F32, tag="pm")
            for half in range(2):
                for qk in range(2):
                    h1v = h1s[qk][:].rearrange("p (j h c) -> p j h c",
                                               j=2, h=2)
                    nc.tensor.matmul(
                        p2[64 * half:64 * half + 64,
                           512 * qk:512 * qk + 512]
                        .rearrange("p (j c) -> p j c", j=2),
                        w2[:], h1v[:, :, half:half + 1, :])
            h2 = sb.tile([128, B], F32R, tag="h2")
            nc.vector.tensor_scalar(
                h2[:], p2[:], b2p[:], 0.0, ALU.add, ALU.max)
            # L3 block-diag f32r over both qi
            p3 = psm.tile([128, B], F32, tag="pm")
            nc.tensor.matmul(p3[:, 0:512], w3p[:], h2[:, 0:512])
            nc.tensor.matmul(p3[:, 512:1024], w3p[:], h2[:, 512:1024])
            h3 = sb.tile([128, B], F32R, tag="h3")
            nc.vector.tensor_scalar(
                h3[:], p3[:], b3p[:], 0.0, ALU.add, ALU.max)
            # L4 block-diag f32r
            p4 = psm.tile([128, B], F32, tag="pm")
            nc.tensor.matmul(p4[:, 0:512], w4p[:], h3[:, 0:512])
            nc.tensor.matmul(p4[:, 512:1024], w4p[:], h3[:, 512:1024])
            h4 = sb.tile([128, B], F32R, tag="h4")
            nc.scalar.activation(h4[:], p4[:], AF.Relu, bias=b4p[:],
                                 scale=1.0)
            # L5 f32r [128, 2] -> [2, 1024]: partition 0 = top rows,
            # partition 1 = +256 rows; cols split by qi
            p5 = psm.tile([2, B], F32, tag="pm")
            nc.tensor.matmul(p5[:, 0:512], w5p[:], h4[:, 0:512])
            nc.tensor.matmul(p5[:, 512:1024], w5p[:], h4[:, 512:1024])
            stage = sb.tile([2, B], F32, tag="stage")
            nc.scalar.activation(stage[:], p5[:], AF.Identity,
                                 bias=b5[0:2, :], scale=1.0)
            # one DMA, scrambled layout: out_raw[qi, p*512 + j*256 + c]
            src = stage[0:2, :].rearrange("p (k c) -> p k c", k=2)
            dst = d_out.ap().rearrange("k (p c) -> p k c", p=2) \
                [:, ds(qa, 2), :]
            nc.sync.dma_start(dst, src)

        for _rep in range(REPEAT):
            if USE_LOOP:
                with tc.For_i(0, QPC_EFF, 2) as i:
                    body(i)
            else:
                for t in range(QPC_EFF // 2):
                    body(2 * t)

    nc.compile()
    return nc


def make_in_maps(**inputs):
    x = np.asarray(inputs["x"], dtype=np.float32)
    y = np.asarray(inputs["y"], dtype=np.float32)
    Ws = [np.asarray(inputs[f"W{i}"], dtype=np.float32) for i in range(6)]
    bs = [np.asarray(inputs[f"b{i}"], dtype=np.float32) for i in range(6)]

    w3p = np.zeros((128, 128), np.float32)
    w3p[0:64, 0:64] = Ws[3]
    w3p[64:128, 64:128] = Ws[3]
    w4p = np.zeros((128, 128), np.float32)
    w4p[0:64, 0:64] = Ws[4]
    w4p[64:128, 64:128] = Ws[4]
    w5p = np.zeros((128, 2), np.float32)
    w5p[0:64, 0] = Ws[5][:, 0]
    w5p[64:128, 1] = Ws[5][:, 0]

    base = {
        "xT": round_f32r(x.T),
        "w0x": round_f32r(Ws[0][0:DX]),
        "w0y": round_f32r(Ws[0][DX:]),
        "w1": round_f32r(Ws[1]),
        "w2": Ws[2].astype(np.float16),
        "w3p": round_f32r(w3p),
        "w4p": round_f32r(w4p),
        "w5p": round_f32r(w5p),
        "b0": bs[0],
        "b1": bs[1],
        "b2p": np.concatenate([bs[2], bs[2]]),
        "b3p": np.concatenate([bs[3], bs[3]]),
        "b4p": np.concatenate([bs[4], bs[4]]),
        "b5": np.full(128, bs[5][0], np.float32),
    }
    in_maps = []
    for c in range(NCORES):
        m = dict(base)
        m["yT"] = round_f32r(y[c * QPC:(c + 1) * QPC].T)
        in_maps.append(m)
    return in_maps


def kernel(**inputs):
    in_maps = make_in_maps(**inputs)
    if "nc" not in _cache:
        _cache["nc"] = build_nc()
    res = None
    for attempt in range(3):
        try:
            res = run_bass_kernel_spmd(_cache["nc"], in_maps,
                                       core_ids=list(range(NCORES)))
            break
        except Exception:
            # transient NRT_EXEC_UNIT_UNRECOVERABLE wedges recover on retry
            if attempt == 2:
                raise
            import time
            time.sleep(5)
    raw = np.concatenate([res.results[c]["out"] for c in range(NCORES)], axis=0)
    S = raw.reshape(B, 2, 2, 256).transpose(0, 2, 1, 3).reshape(B, B)
    return np.ascontiguousarray(S.T)


if __name__ == "__main__":
    rng = np.random.default_rng(0)
    inputs = {"x": rng.standard_normal((B, DX), dtype=np.float32),
              "y": rng.standard_normal((B, DY), dtype=np.float32)}
    dims = [DX + DY, 128, 128, 64, 64, 64, 1]
    for i in range(6):
        s = np.sqrt(2.0 / (dims[i] + dims[i + 1])).astype(np.float32)
        inputs[f"W{i}"] = rng.standard_normal((dims[i], dims[i + 1]),
                                              dtype=np.float32) * s
        inputs[f"b{i}"] = rng.standard_normal(dims[i + 1]).astype(np.float32) * 0.1
    out = kernel(**inputs)
    h = np.concatenate([np.broadcast_to(inputs["x"][None], (B, B, DX)),
                        np.broadcast_to(inputs["y"][:, None], (B, B, DY))],
                       axis=2).reshape(B * B, DX + DY)
    for i in range(6):
        h = h @ inputs[f"W{i}"] + inputs[f"b{i}"]
        if i < 5:
            h = np.maximum(h, 0)
    ref = h.reshape(B, B).T
    err = np.abs(out - ref).max() / np.abs(ref).max()
    print(f"self-check relerr: {err:.3e}")



# revision 5
# speedup vs baseline: 97.7784x; 4.3395x over previous
"""ConcatCritic all-pairs MLP kernel for 8 trn2 NeuronCores.

final[p, q] = MLP(concat(x[p], y[q])) for B=1024 pairs each way;
MLP layers 128->128->128->64->64->64->1, relu on hidden layers.

Sharding: core d owns y rows [d*128, (d+1)*128) and all of x, producing a
[128, 1024] block S_d[qi, p] = g(x[p], y[d*128+qi]); the host concatenates
to S [1024, 1024] and returns S.T.

Per-core dataflow (feature-major: features on partitions, pair-rows stream
as the matmul moving dimension; all matmuls fp16 inputs / fp32 accumulate):
  - Layer 0 factorizes: concat(x,y) @ W0 = x @ W0[:64] + y @ W0[64:].
    U = (x @ W0x + b0)^T [128, 1024] and V = (y_d @ W0y)^T [128, 128] are
    computed once; per qi, h0 = relu(U + V[:, qi]) is one DVE tensor_scalar.
  - L1 (128->128): 2 matmuls N=512 -> psum [128, 1024]; ACT relu -> h1 fp16.
  - L2 (128->64): two col-tiled matmuls pack rows pairwise into one psum
    bank: [0:64, c] = row of each even 256-block, [64:128, c] = +256 row ->
    packed [128, 512]; DVE relu -> h2.
  - L3, L4 (64->64): block-diagonal [128, 128] weights keep the packed
    layout at full array width; DVE relu / ACT relu -> h3, h4.
  - L5 (64->1): [128, 32] zero-padded weights at col positions 0/32/64/96
    for 4 consecutive qi -> psum [128, 512]; one ACT copy (+b5) per 4 qi;
    one descrambling DMA per qi to S[qi, :].
"""
import os
import sys

sys.path.insert(0, "/opt/trn_rl_repo")

import numpy as np
from contextlib import ExitStack

import concourse.bass as bass
import concourse.mybir as mybir
import concourse.tile as tile
from concourse import bacc
from concourse.bass import ds
from concourse.bass_utils import run_bass_kernel_spmd

F32 = mybir.dt.float32
FP16 = mybir.dt.float16
F32R = mybir.dt.float32r
AF = mybir.ActivationFunctionType
ALU = mybir.AluOpType

B = 1024
DX = 64
DY = 64
NCORES = 8
QPC = B // NCORES  # 128 y-rows per core
REPEAT = int(os.environ.get("KERNEL_REPEAT", "1"))
QPC_EFF = int(os.environ.get("KERNEL_QPC", str(QPC)))
USE_LOOP = int(os.environ.get("KERNEL_LOOP", "1"))

_cache = {}


def round_f32r(a):
    """Round fp32 ndarray to the f32r grid (11 explicit mantissa bits, RNE)."""
    u = np.ascontiguousarray(a, dtype=np.float32).view(np.uint32)
    low = u & np.uint32(0x00000FFF)
    base = u & np.uint32(0xFFFFF000)
    lsb = (u >> np.uint32(12)) & np.uint32(1)
    round_up = (low > 0x800) | ((low == 0x800) & (lsb == 1))
    return (base + (round_up.astype(np.uint32) << np.uint32(12))).view(np.float32)


def build_nc():
    nc = bacc.Bacc("TRN2", target_bir_lowering=False, debug=False)

    d_xT = nc.dram_tensor("xT", [DX, B], F32R, kind="ExternalInput")
    d_yT = nc.dram_tensor("yT", [DY, QPC], F32R, kind="ExternalInput")
    d_w0x = nc.dram_tensor("w0x", [DX, 128], F32R, kind="ExternalInput")
    d_w0y = nc.dram_tensor("w0y", [DY, 128], F32R, kind="ExternalInput")
    d_w1 = nc.dram_tensor("w1", [128, 128], F32R, kind="ExternalInput")
    d_w2 = nc.dram_tensor("w2", [128, 64], FP16, kind="ExternalInput")
    d_w3p = nc.dram_tensor("w3p", [128, 128], F32R, kind="ExternalInput")
    d_w4p = nc.dram_tensor("w4p", [128, 128], F32R, kind="ExternalInput")
    d_w5p = nc.dram_tensor("w5p", [128, 2], F32R, kind="ExternalInput")
    d_b0 = nc.dram_tensor("b0", [128], F32, kind="ExternalInput")
    d_b1 = nc.dram_tensor("b1", [128], F32, kind="ExternalInput")
    d_b2p = nc.dram_tensor("b2p", [128], F32, kind="ExternalInput")
    d_b3p = nc.dram_tensor("b3p", [128], F32, kind="ExternalInput")
    d_b4p = nc.dram_tensor("b4p", [128], F32, kind="ExternalInput")
    d_b5 = nc.dram_tensor("b5", [128], F32, kind="ExternalInput")
    d_out = nc.dram_tensor("out", [QPC, B], F32, kind="ExternalOutput")

    with tile.TileContext(nc) as tc, ExitStack() as ctx:
        const = ctx.enter_context(tc.tile_pool(name="const", bufs=1))
        sb = ctx.enter_context(tc.tile_pool(name="sb", bufs=2))
        ps1 = ctx.enter_context(tc.tile_pool(name="ps1", bufs=2, space="PSUM"))
        psm = ctx.enter_context(tc.tile_pool(name="psm", bufs=2, space="PSUM"))

        # ---- load constants -------------------------------------------------
        xT = const.tile([DX, B], F32R)
        yT = const.tile([DY, QPC], F32R)
        w0x = const.tile([DX, 128], F32R)
        w0y = const.tile([DY, 128], F32R)
        w1 = const.tile([128, 128], F32R)
        w2 = const.tile([128, 64], FP16)
        w3p = const.tile([128, 128], F32R)
        w4p = const.tile([128, 128], F32R)
        w5p = const.tile([128, 2], F32R)
        b0 = const.tile([128, 1], F32)
        b1 = const.tile([128, 1], F32)
        b2p = const.tile([128, 1], F32)
        b3p = const.tile([128, 1], F32)
        b4p = const.tile([128, 1], F32)
        b5 = const.tile([128, 1], F32)
        for t, d in [(xT, d_xT), (yT, d_yT), (w0x, d_w0x), (w0y, d_w0y),
                     (w1, d_w1), (w2, d_w2), (w3p, d_w3p), (w4p, d_w4p),
                     (w5p, d_w5p)]:
            nc.sync.dma_start(t[:], d.ap())
        for t, d in [(b0, d_b0), (b1, d_b1), (b2p, d_b2p), (b3p, d_b3p),
                     (b4p, d_b4p), (b5, d_b5)]:
            nc.sync.dma_start(t[:], d.ap()[:, None])

        # ---- preamble: U = (x @ W0x + b0)^T fp16, V = (y @ W0y)^T f32 ------
        U = const.tile([128, B], F32R)
        V = const.tile([128, QPC], F32)
        pU = ps1.tile([128, B], F32, tag="p1")
        nc.tensor.matmul(pU[:, 0:512], w0x[:], xT[:, 0:512])
        nc.tensor.matmul(pU[:, 512:1024], w0x[:], xT[:, 512:1024])
        nc.scalar.activation(U[:], pU[:], AF.Identity, bias=b0[:], scale=1.0)
        pV = psm.tile([128, QPC], F32, tag="pm")
        nc.tensor.matmul(pV[:], w0y[:], yT[:])
        nc.scalar.copy(V[:], pV[:])

        # ---- main loop: batches of 2 qi ------------------------------------
        def body(qa):
            """qa: first qi of the batch; int (unrolled) or ScalarValue."""
            # h0 = relu(U + V[:, qi]) per qi, then L1 + relu1 per qi
            h1s = []
            for qk in range(2):
                h0 = sb.tile([128, B], F32R, tag="h0")
                nc.vector.tensor_scalar(
                    h0[:], U[:], V[:, ds(qa + qk, 1)], 0.0, ALU.add, ALU.max)
                p1 = ps1.tile([128, B], F32, tag="p1")
                nc.tensor.matmul(p1[:, 0:512], w1[:], h0[:, 0:512])
                nc.tensor.matmul(p1[:, 512:1024], w1[:], h0[:, 512:1024])
                h1 = sb.tile([128, B], FP16, tag="h1")
                nc.scalar.activation(h1[:], p1[:], AF.Relu, bias=b1[:],
                                     scale=1.0)
                h1s.append(h1)
            # L2 fp16 col-tiled pack, both qi into one [128, 1024] psum:
            # cols [qk*512 + j*256 + c] <- h1 row j*512 + (half? 256:0) + c
            p2 = psm.tile([128, B], F32, tag="pm")
            for half in range(2):
                for qk in range(2):
                    h1v = h1s[qk][:].rearrange("p (j h c) -> p j h c",
                                               j=2, h=2)
                    nc.tensor.matmul(
                        p2[64 * half:64 * half + 64,
                           512 * qk:512 * qk + 512]
                        .rearrange("p (j c) -> p j c", j=2),
                        w2[:], h1v[:, :, half:half + 1, :])
            h2 = sb.tile([128, B], F32R, tag="h2")
            nc.vector.tensor_scalar(
                h2[:], p2[:], b2p[:], 0.0, ALU.add, ALU.max)
            # L3 block-diag f32r over both qi
            p3 = psm.tile([128, B], F32, tag="pm")
            nc.tensor.matmul(p3[:, 0:512], w3p[:], h2[:, 0:512])
            nc.tensor.matmul(p3[:, 512:1024], w3p[:], h2[:, 512:1024])
            h3 = sb.tile([128, B], F32R, tag="h3")
            nc.vector.tensor_scalar(
                h3[:], p3[:], b3p[:], 0.0, ALU.add, ALU.max)
            # L4 block-diag f32r
            p4 = psm.tile([128, B], F32, tag="pm")
            nc.tensor.matmul(p4[:, 0:512], w4p[:], h3[:, 0:512])
            nc.tensor.matmul(p4[:, 512:1024], w4p[:], h3[:, 512:1024])
            h4 = sb.tile([128, B], F32R, tag="h4")
            nc.scalar.activation(h4[:], p4[:], AF.Relu, bias=b4p[:],
                                 scale=1.0)
            # L5 f32r [128, 2] -> [2, 1024]: partition 0 = top rows,
            # partition 1 = +256 rows; cols split by qi
            p5 = psm.tile([2, B], F32, tag="pm")
            nc.tensor.matmul(p5[:, 0:512], w5p[:], h4[:, 0:512])
            nc.tensor.matmul(p5[:, 512:1024], w5p[:], h4[:, 512:1024])
            stage = sb.tile([2, B], F32, tag="stage")
            nc.scalar.activation(stage[:], p5[:], AF.Identity,
                                 bias=b5[0:2, :], scale=1.0)
            # one DMA, scrambled layout: out_raw[qi, p*512 + j*256 + c]
            src = stage[0:2, :].rearrange("p (k c) -> p k c", k=2)
            dst = d_out.ap().rearrange("k (p c) -> p k c", p=2) \
                [:, ds(qa, 2), :]
            nc.sync.dma_start(dst, src)

        if USE_LOOP:
            # repeat on-device: NEFF size is independent of REPEAT, so
            # repeat-count wall-clock deltas isolate true device exec time
            with tc.For_i(0, REPEAT) as _r:
                with tc.For_i(0, QPC_EFF, 2) as i:
                    body(i)
        else:
            for _rep in range(REPEAT):
                for t in range(QPC_EFF // 2):
                    body(2 * t)

    nc.compile()
    return nc


def make_in_maps(**inputs):
    x = np.asarray(inputs["x"], dtype=np.float32)
    y = np.asarray(inputs["y"], dtype=np.float32)
    Ws = [np.asarray(inputs[f"W{i}"], dtype=np.float32) for i in range(6)]
    bs = [np.asarray(inputs[f"b{i}"], dtype=np.float32) for i in range(6)]

    w3p = np.zeros((128, 128), np.float32)
    w3p[0:64, 0:64] = Ws[3]
    w3p[64:128, 64:128] = Ws[3]
    w4p = np.zeros((128, 128), np.float32)
    w4p[0:64, 0:64] = Ws[4]
    w4p[64:128, 64:128] = Ws[4]
    w5p = np.zeros((128, 2), np.float32)
    w5p[0:64, 0] = Ws[5][:, 0]
    w5p[64:128, 1] = Ws[5][:, 0]

    base = {
        "xT": round_f32r(x.T),
        "w0x": round_f32r(Ws[0][0:DX]),
        "w0y": round_f32r(Ws[0][DX:]),
        "w1": round_f32r(Ws[1]),
        "w2": Ws[2].astype(np.float16),
        "w3p": round_f32r(w3p),
        "w4p": round_f32r(w4p),
        "w5p": round_f32r(w5p),
        "b0": bs[0],
        "b1": bs[1],
        "b2p": np.concatenate([bs[2], bs[2]]),
        "b3p": np.concatenate([bs[3], bs[3]]),
        "b4p": np.concatenate([bs[4], bs[4]]),
        "b5": np.full(128, bs[5][0], np.float32),
    }
    in_maps = []
    for c in range(NCORES):
        m = dict(base)
        m["yT"] = round_f32r(y[c * QPC:(c + 1) * QPC].T)
        in_maps.append(m)
    return in_maps


def kernel(**inputs):
    in_maps = make_in_maps(**inputs)
    if "nc" not in _cache:
        _cache["nc"] = build_nc()
    res = None
    for attempt in range(3):
        try:
            res = run_bass_kernel_spmd(_cache["nc"], in_maps,
                                       core_ids=list(range(NCORES)))
            break
        except Exception:
            # transient NRT_EXEC_UNIT_UNRECOVERABLE wedges recover on retry
            if attempt == 2:
                raise
            import time
            time.sleep(5)
    raw = np.concatenate([res.results[c]["out"] for c in range(NCORES)], axis=0)
    S = raw.reshape(B, 2, 2, 256).transpose(0, 2, 1, 3).reshape(B, B)
    return np.ascontiguousarray(S.T)


if __name__ == "__main__":
    rng = np.random.default_rng(0)
    inputs = {"x": rng.standard_normal((B, DX), dtype=np.float32),
              "y": rng.standard_normal((B, DY), dtype=np.float32)}
    dims = [DX + DY, 128, 128, 64, 64, 64, 1]
    for i in range(6):
        s = np.sqrt(2.0 / (dims[i] + dims[i + 1])).astype(np.float32)
        inputs[f"W{i}"] = rng.standard_normal((dims[i], dims[i + 1]),
                                              dtype=np.float32) * s
        inputs[f"b{i}"] = rng.standard_normal(dims[i + 1]).astype(np.float32) * 0.1
    out = kernel(**inputs)
    h = np.concatenate([np.broadcast_to(inputs["x"][None], (B, B, DX)),
                        np.broadcast_to(inputs["y"][:, None], (B, B, DY))],
                       axis=2).reshape(B * B, DX + DY)
    for i in range(6):
        h = h @ inputs[f"W{i}"] + inputs[f"b{i}"]
        if i < 5:
            h = np.maximum(h, 0)
    ref = h.reshape(B, B).T
    err = np.abs(out - ref).max() / np.abs(ref).max()
    print(f"self-check relerr: {err:.3e}")



# revision 11
# speedup vs baseline: 282.3363x; 2.8875x over previous
"""ConcatCritic all-pairs MLP kernel for 8 trn2 NeuronCores.

final[p, q] = MLP(concat(x[p], y[q])) for B=1024 pairs each way;
MLP layers 128->128->128->64->64->64->1, relu on hidden layers.

Sharding: core d owns y rows [d*128, (d+1)*128) and all of x, producing a
[128, 1024] block S_d[qi, p] = g(x[p], y[d*128+qi]); the host concatenates
to S [1024, 1024] and returns S.T.

Per-core dataflow (feature-major: features on partitions, pair-rows stream
as the matmul moving dimension; all matmuls fp16 inputs / fp32 accumulate):
  - Layer 0 factorizes: concat(x,y) @ W0 = x @ W0[:64] + y @ W0[64:].
    U = (x @ W0x + b0)^T [128, 1024] and V = (y_d @ W0y)^T [128, 128] are
    computed once; per qi, h0 = relu(U + V[:, qi]) is one DVE tensor_scalar.
  - L1 (128->128): 2 matmuls N=512 -> psum [128, 1024]; ACT relu -> h1 fp16.
  - L2 (128->64): two col-tiled matmuls pack rows pairwise into one psum
    bank: [0:64, c] = row of each even 256-block, [64:128, c] = +256 row ->
    packed [128, 512]; DVE relu -> h2.
  - L3, L4 (64->64): block-diagonal [128, 128] weights keep the packed
    layout at full array width; DVE relu / ACT relu -> h3, h4.
  - L5 (64->1): [128, 32] zero-padded weights at col positions 0/32/64/96
    for 4 consecutive qi -> psum [128, 512]; one ACT copy (+b5) per 4 qi;
    one descrambling DMA per qi to S[qi, :].

Loop structure: the qi loop runs as a hardware For_i (16 bodies = 32 qi per
back-edge), nested inside an on-device REPEAT For_i, so the NEFF holds one
copy of the body regardless of repeat count. This matters doubly here: the
axon per-call overhead scales with static NEFF size (~tens of us per
instruction), and repeat-count wall-clock deltas then cancel everything
except true per-pass device exec. The final bias-add runs on DVE (not ACT)
so ACT only ever uses the Relu table -- no per-iteration table reloads.
"""
import os
import sys

sys.path.insert(0, "/opt/trn_rl_repo")

import numpy as np
from contextlib import ExitStack

import concourse.bass as bass
import concourse.mybir as mybir
import concourse.tile as tile
from concourse import bacc
from concourse.bass import ds
from concourse.bass_utils import run_bass_kernel_spmd

F32 = mybir.dt.float32
FP16 = mybir.dt.float16
F32R = mybir.dt.float32r
AF = mybir.ActivationFunctionType
ALU = mybir.AluOpType

B = 1024
DX = 64
DY = 64
NCORES = 8
QPC = B // NCORES  # 128 y-rows per core
REPEAT = int(os.environ.get("KERNEL_REPEAT", "1"))
QPC_EFF = int(os.environ.get("KERNEL_QPC", str(QPC)))
USE_LOOP = int(os.environ.get("KERNEL_LOOP", "1"))
UNROLL = int(os.environ.get("KERNEL_UNROLL", "16"))
STAGGER = int(os.environ.get("KERNEL_STAGGER", "0"))
STAGE_DVE = int(os.environ.get("KERNEL_STAGE_DVE", "1"))

_cache = {}


def round_f32r(a):
    """Round fp32 ndarray to the f32r grid (11 explicit mantissa bits, RNE)."""
    u = np.ascontiguousarray(a, dtype=np.float32).view(np.uint32)
    low = u & np.uint32(0x00000FFF)
    base = u & np.uint32(0xFFFFF000)
    lsb = (u >> np.uint32(12)) & np.uint32(1)
    round_up = (low > 0x800) | ((low == 0x800) & (lsb == 1))
    return (base + (round_up.astype(np.uint32) << np.uint32(12))).view(np.float32)


def build_nc():
    nc = bacc.Bacc("TRN2", target_bir_lowering=False, debug=False)

    d_xT = nc.dram_tensor("xT", [DX, B], F32R, kind="ExternalInput")
    d_yT = nc.dram_tensor("yT", [DY, QPC], F32R, kind="ExternalInput")
    d_w0x = nc.dram_tensor("w0x", [DX, 128], F32R, kind="ExternalInput")
    d_w0y = nc.dram_tensor("w0y", [DY, 128], F32R, kind="ExternalInput")
    d_w1 = nc.dram_tensor("w1", [128, 128], F32R, kind="ExternalInput")
    d_w2 = nc.dram_tensor("w2", [128, 64], FP16, kind="ExternalInput")
    d_w3p = nc.dram_tensor("w3p", [128, 128], F32R, kind="ExternalInput")
    d_w4p = nc.dram_tensor("w4p", [128, 128], F32R, kind="ExternalInput")
    d_w5p = nc.dram_tensor("w5p", [128, 2], F32R, kind="ExternalInput")
    d_b0 = nc.dram_tensor("b0", [128], F32, kind="ExternalInput")
    d_b1 = nc.dram_tensor("b1", [128], F32, kind="ExternalInput")
    d_b2p = nc.dram_tensor("b2p", [128], F32, kind="ExternalInput")
    d_b3p = nc.dram_tensor("b3p", [128], F32, kind="ExternalInput")
    d_b4p = nc.dram_tensor("b4p", [128], F32, kind="ExternalInput")
    d_b5 = nc.dram_tensor("b5", [128], F32, kind="ExternalInput")
    d_out = nc.dram_tensor("out", [QPC, B], F32, kind="ExternalOutput")

    with tile.TileContext(nc) as tc, ExitStack() as ctx:
        const = ctx.enter_context(tc.tile_pool(name="const", bufs=1))
        sb = ctx.enter_context(tc.tile_pool(name="sb", bufs=2))
        ps1 = ctx.enter_context(tc.tile_pool(name="ps1", bufs=2, space="PSUM"))
        psm = ctx.enter_context(tc.tile_pool(name="psm", bufs=2, space="PSUM"))

        # ---- load constants -------------------------------------------------
        xT = const.tile([DX, B], F32R)
        yT = const.tile([DY, QPC], F32R)
        w0x = const.tile([DX, 128], F32R)
        w0y = const.tile([DY, 128], F32R)
        w1 = const.tile([128, 128], F32R)
        w2 = const.tile([128, 64], FP16)
        w3p = const.tile([128, 128], F32R)
        w4p = const.tile([128, 128], F32R)
        w5p = const.tile([128, 2], F32R)
        b0 = const.tile([128, 1], F32)
        b1 = const.tile([128, 1], F32)
        b2p = const.tile([128, 1], F32)
        b3p = const.tile([128, 1], F32)
        b4p = const.tile([128, 1], F32)
        b5 = const.tile([128, 1], F32)
        for t, d in [(xT, d_xT), (yT, d_yT), (w0x, d_w0x), (w0y, d_w0y),
                     (w1, d_w1), (w2, d_w2), (w3p, d_w3p), (w4p, d_w4p),
                     (w5p, d_w5p)]:
            nc.sync.dma_start(t[:], d.ap())
        for t, d in [(b0, d_b0), (b1, d_b1), (b2p, d_b2p), (b3p, d_b3p),
                     (b4p, d_b4p), (b5, d_b5)]:
            nc.sync.dma_start(t[:], d.ap()[:, None])

        # ---- preamble: U = (x @ W0x + b0)^T fp16, V = (y @ W0y)^T f32 ------
        U = const.tile([128, B], F32R)
        V = const.tile([128, QPC], F32)
        pU = ps1.tile([128, B], F32, tag="p1")
        nc.tensor.matmul(pU[:, 0:512], w0x[:], xT[:, 0:512])
        nc.tensor.matmul(pU[:, 512:1024], w0x[:], xT[:, 512:1024])
        nc.scalar.activation(U[:], pU[:], AF.Identity, bias=b0[:], scale=1.0)
        pV = psm.tile([128, QPC], F32, tag="pm")
        nc.tensor.matmul(pV[:], w0y[:], yT[:])
        nc.scalar.copy(V[:], pV[:])

        # ---- main loop: batches of 2 qi ------------------------------------
        def body(qa):
            """qa: first qi of the batch; int (unrolled) or ScalarValue."""
            # h0 = relu(U + V[:, qi]) per qi, then L1 + relu1 per qi
            h1s = []
            for qk in range(2):
                h0 = sb.tile([128, B], F32R, tag="h0")
                nc.vector.tensor_scalar(
                    h0[:], U[:], V[:, ds(qa + qk, 1)], 0.0, ALU.add, ALU.max)
                p1 = ps1.tile([128, B], F32, tag="p1")
                nc.tensor.matmul(p1[:, 0:512], w1[:], h0[:, 0:512])
                nc.tensor.matmul(p1[:, 512:1024], w1[:], h0[:, 512:1024])
                h1 = sb.tile([128, B], FP16, tag="h1")
                nc.scalar.activation(h1[:], p1[:], AF.Relu, bias=b1[:],
                                     scale=1.0)
                h1s.append(h1)
            # L2 fp16 col-tiled pack, both qi into one [128, 1024] psum:
            # cols [qk*512 + j*256 + c] <- h1 row j*512 + (half? 256:0) + c
            p2 = psm.tile([128, B], F32, tag="pm")
            for half in range(2):
                for qk in range(2):
                    h1v = h1s[qk][:].rearrange("p (j h c) -> p j h c",
                                               j=2, h=2)
                    nc.tensor.matmul(
                        p2[64 * half:64 * half + 64,
                           512 * qk:512 * qk + 512]
                        .rearrange("p (j c) -> p j c", j=2),
                        w2[:], h1v[:, :, half:half + 1, :])
            h2 = sb.tile([128, B], F32R, tag="h2")
            nc.vector.tensor_scalar(
                h2[:], p2[:], b2p[:], 0.0, ALU.add, ALU.max)
            # L3 block-diag f32r over both qi
            p3 = psm.tile([128, B], F32, tag="pm")
            nc.tensor.matmul(p3[:, 0:512], w3p[:], h2[:, 0:512])
            nc.tensor.matmul(p3[:, 512:1024], w3p[:], h2[:, 512:1024])
            h3 = sb.tile([128, B], F32R, tag="h3")
            nc.vector.tensor_scalar(
                h3[:], p3[:], b3p[:], 0.0, ALU.add, ALU.max)
            # L4 block-diag f32r
            p4 = psm.tile([128, B], F32, tag="pm")
            nc.tensor.matmul(p4[:, 0:512], w4p[:], h3[:, 0:512])
            nc.tensor.matmul(p4[:, 512:1024], w4p[:], h3[:, 512:1024])
            h4 = sb.tile([128, B], F32R, tag="h4")
            nc.scalar.activation(h4[:], p4[:], AF.Relu, bias=b4p[:],
                                 scale=1.0)
            # L5 f32r [128, 2] -> [2, 1024]: partition 0 = top rows,
            # partition 1 = +256 rows; cols split by qi
            p5 = psm.tile([2, B], F32, tag="pm")
            nc.tensor.matmul(p5[:, 0:512], w5p[:], h4[:, 0:512])
            nc.tensor.matmul(p5[:, 512:1024], w5p[:], h4[:, 512:1024])
            stage = sb.tile([2, B], F32, tag="stage")
            if STAGE_DVE:
                # keep ACT pure-Relu (no Identity table swap); bias on DVE
                nc.vector.tensor_scalar(stage[:], p5[:], b5[0:2, :], 0.0,
                                        ALU.add, ALU.bypass)
            else:
                nc.scalar.activation(stage[:], p5[:], AF.Identity,
                                     bias=b5[0:2, :], scale=1.0)
            # one DMA, scrambled layout: out_raw[qi, p*512 + j*256 + c]
            src = stage[0:2, :].rearrange("p (k c) -> p k c", k=2)
            dst = d_out.ap().rearrange("k (p c) -> p k c", p=2) \
                [:, ds(qa, 2), :]
            nc.sync.dma_start(dst, src)

        if USE_LOOP:
            # repeat on-device: NEFF size is independent of REPEAT, so
            # repeat-count wall-clock deltas isolate true device exec time
            with tc.For_i(0, REPEAT) as _r:
                with tc.For_i(0, QPC_EFF, 2 * UNROLL,
                              staggered_reset=bool(STAGGER)) as i:
                    for u in range(UNROLL):
                        body(i + 2 * u)
        else:
            for _rep in range(REPEAT):
                for t in range(QPC_EFF // 2):
                    body(2 * t)

    nc.compile()
    return nc


def make_in_maps(**inputs):
    x = np.asarray(inputs["x"], dtype=np.float32)
    y = np.asarray(inputs["y"], dtype=np.float32)
    Ws = [np.asarray(inputs[f"W{i}"], dtype=np.float32) for i in range(6)]
    bs = [np.asarray(inputs[f"b{i}"], dtype=np.float32) for i in range(6)]

    w3p = np.zeros((128, 128), np.float32)
    w3p[0:64, 0:64] = Ws[3]
    w3p[64:128, 64:128] = Ws[3]
    w4p = np.zeros((128, 128), np.float32)
    w4p[0:64, 0:64] = Ws[4]
    w4p[64:128, 64:128] = Ws[4]
    w5p = np.zeros((128, 2), np.float32)
    w5p[0:64, 0] = Ws[5][:, 0]
    w5p[64:128, 1] = Ws[5][:, 0]

    base = {
        "xT": round_f32r(x.T),
        "w0x": round_f32r(Ws[0][0:DX]),
        "w0y": round_f32r(Ws[0][DX:]),
        "w1": round_f32r(Ws[1]),
        "w2": Ws[2].astype(np.float16),
        "w3p": round_f32r(w3p),
        "w4p": round_f32r(w4p),
        "w5p": round_f32r(w5p),
        "b0": bs[0],
        "b1": bs[1],
        "b2p": np.concatenate([bs[2], bs[2]]),
        "b3p": np.concatenate([bs[3], bs[3]]),
        "b4p": np.concatenate([bs[4], bs[4]]),
        "b5": np.full(128, bs[5][0], np.float32),
    }
    in_maps = []
    for c in range(NCORES):
        m = dict(base)
        m["yT"] = round_f32r(y[c * QPC:(c + 1) * QPC].T)
        in_maps.append(m)
    return in_maps


def kernel(**inputs):
    in_maps = make_in_maps(**inputs)
    if "nc" not in _cache:
        _cache["nc"] = build_nc()
    res = None
    for attempt in range(3):
        try:
            res = run_bass_kernel_spmd(_cache["nc"], in_maps,
                                       core_ids=list(range(NCORES)))
            break
        except Exception:
            # transient NRT_EXEC_UNIT_UNRECOVERABLE wedges recover on retry
            if attempt == 2:
                raise
            import time
            time.sleep(5)
    raw = np.concatenate([res.results[c]["out"] for c in range(NCORES)], axis=0)
    S = raw.reshape(B, 2, 2, 256).transpose(0, 2, 1, 3).reshape(B, B)
    return np.ascontiguousarray(S.T)


if __name__ == "__main__":
    rng = np.random.default_rng(0)
    inputs = {"x": rng.standard_normal((B, DX), dtype=np.float32),
              "y": rng.standard_normal((B, DY), dtype=np.float32)}
    dims = [DX + DY, 128, 128, 64, 64, 64, 1]
    for i in range(6):
        s = np.sqrt(2.0 / (dims[i] + dims[i + 1])).astype(np.float32)
        inputs[f"W{i}"] = rng.standard_normal((dims[i], dims[i + 1]),
                                              dtype=np.float32) * s
        inputs[f"b{i}"] = rng.standard_normal(dims[i + 1]).astype(np.float32) * 0.1
    out = kernel(**inputs)
    h = np.concatenate([np.broadcast_to(inputs["x"][None], (B, B, DX)),
                        np.broadcast_to(inputs["y"][:, None], (B, B, DY))],
                       axis=2).reshape(B * B, DX + DY)
    for i in range(6):
        h = h @ inputs[f"W{i}"] + inputs[f"b{i}"]
        if i < 5:
            h = np.maximum(h, 0)
    ref = h.reshape(B, B).T
    err = np.abs(out - ref).max() / np.abs(ref).max()
    print(f"self-check relerr: {err:.3e}")



# revision 15
# speedup vs baseline: 858.0096x; 3.0390x over previous
"""ConcatCritic all-pairs MLP kernel for 8 trn2 NeuronCores.

final[p, q] = MLP(concat(x[p], y[q])) for B=1024 pairs each way;
MLP layers 128->128->128->64->64->64->1, relu on hidden layers.

Sharding: core d owns y rows [d*128, (d+1)*128) and all of x, producing a
[128, 1024] block S_d[qi, p] = g(x[p], y[d*128+qi]); the host concatenates
to S [1024, 1024] and returns S.T.

Per-core dataflow (feature-major: features on partitions, pair-rows stream
as the matmul moving dimension; all matmuls fp16 inputs / fp32 accumulate):
  - Layer 0 factorizes: concat(x,y) @ W0 = x @ W0[:64] + y @ W0[64:].
    U = (x @ W0x + b0)^T [128, 1024] and V = (y_d @ W0y)^T [128, 128] are
    computed once; per qi, h0 = relu(U + V[:, qi]) is one DVE tensor_scalar.
  - L1 (128->128): 2 matmuls N=512 -> psum [128, 1024]; ACT relu -> h1 fp16.
  - L2 (128->64): two col-tiled matmuls pack rows pairwise into one psum
    bank: [0:64, c] = row of each even 256-block, [64:128, c] = +256 row ->
    packed [128, 512]; DVE relu -> h2.
  - L3, L4 (64->64): block-diagonal [128, 128] weights keep the packed
    layout at full array width; DVE relu / ACT relu -> h3, h4.
  - L5 (64->1): [128, 32] zero-padded weights at col positions 0/32/64/96
    for 4 consecutive qi -> psum [128, 512]; one ACT copy (+b5) per 4 qi;
    one descrambling DMA per qi to S[qi, :].

Loop structure: the qi loop runs as a hardware For_i (16 bodies = 32 qi per
back-edge), nested inside an on-device REPEAT For_i, so the NEFF holds one
copy of the body regardless of repeat count. This matters doubly here: the
axon per-call overhead scales with static NEFF size (~tens of us per
instruction), and repeat-count wall-clock deltas then cancel everything
except true per-pass device exec. The final bias-add runs on DVE (not ACT)
so ACT only ever uses the Relu table -- no per-iteration table reloads.
"""
import os
import sys

sys.path.insert(0, "/opt/trn_rl_repo")

import numpy as np
from contextlib import ExitStack

import concourse.bass as bass
import concourse.mybir as mybir
import concourse.tile as tile
from concourse import bacc
from concourse.bass import ds
from concourse.bass_utils import run_bass_kernel_spmd

F32 = mybir.dt.float32
FP16 = mybir.dt.float16
F32R = mybir.dt.float32r
AF = mybir.ActivationFunctionType
ALU = mybir.AluOpType

B = 1024
DX = 64
DY = 64
NCORES = 8
QPC = B // NCORES  # 128 y-rows per core
REPEAT = int(os.environ.get("KERNEL_REPEAT", "1"))
QPC_EFF = int(os.environ.get("KERNEL_QPC", str(QPC)))
USE_LOOP = int(os.environ.get("KERNEL_LOOP", "1"))
UNROLL = int(os.environ.get("KERNEL_UNROLL", "16"))
STAGGER = int(os.environ.get("KERNEL_STAGGER", "0"))
STAGE_DVE = int(os.environ.get("KERNEL_STAGE_DVE", "1"))
TAIL_SPLIT = int(os.environ.get("KERNEL_TAIL_SPLIT", "0"))

_cache = {}


def round_f32r(a):
    """Round fp32 ndarray to the f32r grid (11 explicit mantissa bits, RNE)."""
    u = np.ascontiguousarray(a, dtype=np.float32).view(np.uint32)
    low = u & np.uint32(0x00000FFF)
    base = u & np.uint32(0xFFFFF000)
    lsb = (u >> np.uint32(12)) & np.uint32(1)
    round_up = (low > 0x800) | ((low == 0x800) & (lsb == 1))
    return (base + (round_up.astype(np.uint32) << np.uint32(12))).view(np.float32)


def build_nc():
    nc = bacc.Bacc("TRN2", target_bir_lowering=False, debug=False)

    d_xT = nc.dram_tensor("xT", [DX, B], F32R, kind="ExternalInput")
    d_yT = nc.dram_tensor("yT", [DY, QPC], F32R, kind="ExternalInput")
    d_w0x = nc.dram_tensor("w0x", [DX, 128], F32R, kind="ExternalInput")
    d_w0y = nc.dram_tensor("w0y", [DY, 128], F32R, kind="ExternalInput")
    d_w1 = nc.dram_tensor("w1", [128, 128], F32R, kind="ExternalInput")
    d_w2 = nc.dram_tensor("w2", [128, 64], FP16, kind="ExternalInput")
    d_w3p = nc.dram_tensor("w3p", [128, 128], F32R, kind="ExternalInput")
    d_w4p = nc.dram_tensor("w4p", [128, 128], F32R, kind="ExternalInput")
    d_w5p = nc.dram_tensor("w5p", [128, 2], F32R, kind="ExternalInput")
    d_b0 = nc.dram_tensor("b0", [128], F32, kind="ExternalInput")
    d_b1 = nc.dram_tensor("b1", [128], F32, kind="ExternalInput")
    d_b2p = nc.dram_tensor("b2p", [128], F32, kind="ExternalInput")
    d_b3p = nc.dram_tensor("b3p", [128], F32, kind="ExternalInput")
    d_b4p = nc.dram_tensor("b4p", [128], F32, kind="ExternalInput")
    d_b5 = nc.dram_tensor("b5", [128], F32, kind="ExternalInput")
    d_out = nc.dram_tensor("out", [QPC, B], F32, kind="ExternalOutput")

    with tile.TileContext(nc) as tc, ExitStack() as ctx:
        const = ctx.enter_context(tc.tile_pool(name="const", bufs=1))
        sb = ctx.enter_context(tc.tile_pool(name="sb", bufs=2))
        ps1 = ctx.enter_context(tc.tile_pool(name="ps1", bufs=2, space="PSUM"))
        psm = ctx.enter_context(
            tc.tile_pool(name="psm", bufs=(4 if TAIL_SPLIT else 2),
                         space="PSUM"))

        # ---- load constants -------------------------------------------------
        xT = const.tile([DX, B], F32R)
        yT = const.tile([DY, QPC], F32R)
        w0x = const.tile([DX, 128], F32R)
        w0y = const.tile([DY, 128], F32R)
        w1 = const.tile([128, 128], F32R)
        w2 = const.tile([128, 64], FP16)
        w3p = const.tile([128, 128], F32R)
        w4p = const.tile([128, 128], F32R)
        w5p = const.tile([128, 2], F32R)
        b0 = const.tile([128, 1], F32)
        b1 = const.tile([128, 1], F32)
        b2p = const.tile([128, 1], F32)
        b3p = const.tile([128, 1], F32)
        b4p = const.tile([128, 1], F32)
        b5 = const.tile([128, 1], F32)
        for t, d in [(xT, d_xT), (yT, d_yT), (w0x, d_w0x), (w0y, d_w0y),
                     (w1, d_w1), (w2, d_w2), (w3p, d_w3p), (w4p, d_w4p),
                     (w5p, d_w5p)]:
            nc.sync.dma_start(t[:], d.ap())
        for t, d in [(b0, d_b0), (b1, d_b1), (b2p, d_b2p), (b3p, d_b3p),
                     (b4p, d_b4p), (b5, d_b5)]:
            nc.sync.dma_start(t[:], d.ap()[:, None])

        # ---- preamble: U = (x @ W0x + b0)^T fp16, V = (y @ W0y)^T f32 ------
        U = const.tile([128, B], F32R)
        V = const.tile([128, QPC], F32)
        pU = ps1.tile([128, B], F32, tag="p1")
        nc.tensor.matmul(pU[:, 0:512], w0x[:], xT[:, 0:512])
        nc.tensor.matmul(pU[:, 512:1024], w0x[:], xT[:, 512:1024])
        nc.scalar.activation(U[:], pU[:], AF.Identity, bias=b0[:], scale=1.0)
        pV = psm.tile([128, QPC], F32, tag="pm")
        nc.tensor.matmul(pV[:], w0y[:], yT[:])
        nc.scalar.copy(V[:], pV[:])

        # ---- main loop: batches of 2 qi ------------------------------------
        def body(qa):
            """qa: first qi of the batch; int (unrolled) or ScalarValue."""
            # h0 = relu(U + V[:, qi]) per qi, then L1 + relu1 per qi
            h1s = []
            for qk in range(2):
                h0 = sb.tile([128, B], F32R, tag="h0")
                nc.vector.tensor_scalar(
                    h0[:], U[:], V[:, ds(qa + qk, 1)], 0.0, ALU.add, ALU.max)
                p1 = ps1.tile([128, B], F32, tag="p1")
                nc.tensor.matmul(p1[:, 0:512], w1[:], h0[:, 0:512])
                nc.tensor.matmul(p1[:, 512:1024], w1[:], h0[:, 512:1024])
                h1 = sb.tile([128, B], FP16, tag="h1")
                nc.scalar.activation(h1[:], p1[:], AF.Relu, bias=b1[:],
                                     scale=1.0)
                h1s.append(h1)
            if TAIL_SPLIT:
                # per-qi tail on 1-bank [128, 512] psum tiles (psm bufs=4):
                # shorter dep chains let body k+1's L2 start while body k's
                # tail drains, instead of serializing on 2-bank buffer reuse
                stage = sb.tile([2, B], F32, tag="stage")
                for qk in range(2):
                    p2q = psm.tile([128, 512], F32, tag="pm")
                    h1v = h1s[qk][:].rearrange("p (j h c) -> p j h c",
                                               j=2, h=2)
                    for half in range(2):
                        nc.tensor.matmul(
                            p2q[64 * half:64 * half + 64, :]
                            .rearrange("p (j c) -> p j c", j=2),
                            w2[:], h1v[:, :, half:half + 1, :])
                    h2q = sb.tile([128, 512], F32R, tag="h2")
                    nc.vector.tensor_scalar(
                        h2q[:], p2q[:], b2p[:], 0.0, ALU.add, ALU.max)
                    p3q = psm.tile([128, 512], F32, tag="pm")
                    nc.tensor.matmul(p3q[:], w3p[:], h2q[:])
                    h3q = sb.tile([128, 512], F32R, tag="h3")
                    nc.vector.tensor_scalar(
                        h3q[:], p3q[:], b3p[:], 0.0, ALU.add, ALU.max)
                    p4q = psm.tile([128, 512], F32, tag="pm")
                    nc.tensor.matmul(p4q[:], w4p[:], h3q[:])
                    h4q = sb.tile([128, 512], F32R, tag="h4")
                    nc.scalar.activation(h4q[:], p4q[:], AF.Relu,
                                         bias=b4p[:], scale=1.0)
                    p5q = psm.tile([2, 512], F32, tag="pm")
                    nc.tensor.matmul(p5q[:], w5p[:], h4q[:])
                    nc.vector.tensor_scalar(
                        stage[:, 512 * qk:512 * qk + 512], p5q[:],
                        b5[0:2, :], 0.0, ALU.add, ALU.bypass)
                src = stage[0:2, :].rearrange("p (k c) -> p k c", k=2)
                dst = d_out.ap().rearrange("k (p c) -> p k c", p=2) \
                    [:, ds(qa, 2), :]
                nc.sync.dma_start(dst, src)
                return

            # L2 fp16 col-tiled pack, both qi into one [128, 1024] psum:
            # cols [qk*512 + j*256 + c] <- h1 row j*512 + (half? 256:0) + c
            p2 = psm.tile([128, B], F32, tag="pm")
            for half in range(2):
                for qk in range(2):
                    h1v = h1s[qk][:].rearrange("p (j h c) -> p j h c",
                                               j=2, h=2)
                    nc.tensor.matmul(
                        p2[64 * half:64 * half + 64,
                           512 * qk:512 * qk + 512]
                        .rearrange("p (j c) -> p j c", j=2),
                        w2[:], h1v[:, :, half:half + 1, :])
            h2 = sb.tile([128, B], F32R, tag="h2")
            nc.vector.tensor_scalar(
                h2[:], p2[:], b2p[:], 0.0, ALU.add, ALU.max)
            # L3 block-diag f32r over both qi
            p3 = psm.tile([128, B], F32, tag="pm")
            nc.tensor.matmul(p3[:, 0:512], w3p[:], h2[:, 0:512])
            nc.tensor.matmul(p3[:, 512:1024], w3p[:], h2[:, 512:1024])
            h3 = sb.tile([128, B], F32R, tag="h3")
            nc.vector.tensor_scalar(
                h3[:], p3[:], b3p[:], 0.0, ALU.add, ALU.max)
            # L4 block-diag f32r
            p4 = psm.tile([128, B], F32, tag="pm")
            nc.tensor.matmul(p4[:, 0:512], w4p[:], h3[:, 0:512])
            nc.tensor.matmul(p4[:, 512:1024], w4p[:], h3[:, 512:1024])
            h4 = sb.tile([128, B], F32R, tag="h4")
            nc.scalar.activation(h4[:], p4[:], AF.Relu, bias=b4p[:],
                                 scale=1.0)
            # L5 f32r [128, 2] -> [2, 1024]: partition 0 = top rows,
            # partition 1 = +256 rows; cols split by qi
            p5 = psm.tile([2, B], F32, tag="pm")
            nc.tensor.matmul(p5[:, 0:512], w5p[:], h4[:, 0:512])
            nc.tensor.matmul(p5[:, 512:1024], w5p[:], h4[:, 512:1024])
            stage = sb.tile([2, B], F32, tag="stage")
            if STAGE_DVE:
                # keep ACT pure-Relu (no Identity table swap); bias on DVE
                nc.vector.tensor_scalar(stage[:], p5[:], b5[0:2, :], 0.0,
                                        ALU.add, ALU.bypass)
            else:
                nc.scalar.activation(stage[:], p5[:], AF.Identity,
                                     bias=b5[0:2, :], scale=1.0)
            # one DMA, scrambled layout: out_raw[qi, p*512 + j*256 + c]
            src = stage[0:2, :].rearrange("p (k c) -> p k c", k=2)
            dst = d_out.ap().rearrange("k (p c) -> p k c", p=2) \
                [:, ds(qa, 2), :]
            nc.sync.dma_start(dst, src)

        if USE_LOOP:
            # repeat on-device: NEFF size is independent of REPEAT, so
            # repeat-count wall-clock deltas isolate true device exec time
            with tc.For_i(0, REPEAT) as _r:
                with tc.For_i(0, QPC_EFF, 2 * UNROLL,
                              staggered_reset=bool(STAGGER)) as i:
                    for u in range(UNROLL):
                        body(i + 2 * u)
        else:
            for _rep in range(REPEAT):
                for t in range(QPC_EFF // 2):
                    body(2 * t)

    nc.compile()
    return nc


def make_in_maps(**inputs):
    x = np.asarray(inputs["x"], dtype=np.float32)
    y = np.asarray(inputs["y"], dtype=np.float32)
    Ws = [np.asarray(inputs[f"W{i}"], dtype=np.float32) for i in range(6)]
    bs = [np.asarray(inputs[f"b{i}"], dtype=np.float32) for i in range(6)]

    w3p = np.zeros((128, 128), np.float32)
    w3p[0:64, 0:64] = Ws[3]
    w3p[64:128, 64:128] = Ws[3]
    w4p = np.zeros((128, 128), np.float32)
    w4p[0:64, 0:64] = Ws[4]
    w4p[64:128, 64:128] = Ws[4]
    w5p = np.zeros((128, 2), np.float32)
    w5p[0:64, 0] = Ws[5][:, 0]
    w5p[64:128, 1] = Ws[5][:, 0]

    base = {
        "xT": round_f32r(x.T),
        "w0x": round_f32r(Ws[0][0:DX]),
        "w0y": round_f32r(Ws[0][DX:]),
        "w1": round_f32r(Ws[1]),
        "w2": Ws[2].astype(np.float16),
        "w3p": round_f32r(w3p),
        "w4p": round_f32r(w4p),
        "w5p": round_f32r(w5p),
        "b0": bs[0],
        "b1": bs[1],
        "b2p": np.concatenate([bs[2], bs[2]]),
        "b3p": np.concatenate([bs[3], bs[3]]),
        "b4p": np.concatenate([bs[4], bs[4]]),
        "b5": np.full(128, bs[5][0], np.float32),
    }
    in_maps = []
    for c in range(NCORES):
        m = dict(base)
        m["yT"] = round_f32r(y[c * QPC:(c + 1) * QPC].T)
        in_maps.append(m)
    return in_maps


def kernel(**inputs):
    in_maps = make_in_maps(**inputs)
    if "nc" not in _cache:
        _cache["nc"] = build_nc()
    res = None
    for attempt in range(3):
        try:
            res = run_bass_kernel_spmd(_cache["nc"], in_maps,
                                       core_ids=list(range(NCORES)))
            break
        except Exception:
            # transient NRT_EXEC_UNIT_UNRECOVERABLE wedges recover on retry
            if attempt == 2:
                raise
            import time
            time.sleep(5)
    raw = np.concatenate([res.results[c]["out"] for c in range(NCORES)], axis=0)
    S = raw.reshape(B, 2, 2, 256).transpose(0, 2, 1, 3).reshape(B, B)
    return np.ascontiguousarray(S.T)


if __name__ == "__main__":
    rng = np.random.default_rng(0)
    inputs = {"x": rng.standard_normal((B, DX), dtype=np.float32),
              "y": rng.standard_normal((B, DY), dtype=np.float32)}
    dims = [DX + DY, 128, 128, 64, 64, 64, 1]
    for i in range(6):
        s = np.sqrt(2.0 / (dims[i] + dims[i + 1])).astype(np.float32)
        inputs[f"W{i}"] = rng.standard_normal((dims[i], dims[i + 1]),
                                              dtype=np.float32) * s
        inputs[f"b{i}"] = rng.standard_normal(dims[i + 1]).astype(np.float32) * 0.1
    out = kernel(**inputs)
    h = np.concatenate([np.broadcast_to(inputs["x"][None], (B, B, DX)),
                        np.broadcast_to(inputs["y"][:, None], (B, B, DY))],
                       axis=2).reshape(B * B, DX + DY)
    for i in range(6):
        h = h @ inputs[f"W{i}"] + inputs[f"b{i}"]
        if i < 5:
            h = np.maximum(h, 0)
    ref = h.reshape(B, B).T
    err = np.abs(out - ref).max() / np.abs(ref).max()
    print(f"self-check relerr: {err:.3e}")

